# revision 1
# baseline (speedup 1.0000x reference)
"""Trainium2 Bass kernel for nn_Diff_Label01_Loss (masked cosine-similarity loss).

Contract: kernel(labels, datas) takes FULL inputs (labels [8192,2] f32,
datas [8192,4096] f32), returns (total_loss, sim_loss, differ_loss).

Strategy — shard D (columns) across the 8 cores; NO collective:
  Core c owns cols [c*512, (c+1)*512) of datas, in TWO fp8 layouts
  (8.4MB/core total, the HBM-bandwidth floor):
    x_rm [128, 64, 512]   row-tiles (partition p of tile t = row t*128+p)
    xT   [128, 2, 2, 8, 2, 512]  [p, g, h, j, kt, c] = x[h*4096+j*512+c,
                                  (2g+kt)*128+p] — pair-interleaved for
                                  DoubleRow fp8 matmuls
  s0_c   = masked column sum of the core's slice — PE DoubleRow matmuls,
           mask-pair stationary, accumulated in psum[0:1, 0:512]
  m8_c   = fp8(bf16(s0_c * 2^-6)) — ACT cast to bf16, then four K=1
           matmuls spread it onto partitions (psum [128,4]), DVE copies
           (with fp8 cast) into the pair-layout stationary slots
  numer  = x_slice @ m8_c for all 8192 rows — PE DoubleRow, row-half A
           then B through the same psum[0:1, 0:4096]; xT half B ships as
           1.5MB + 0.5MB chunks so the PE chases ingest and only the last
           4 matmuls + a [1,1024]-pair spill sit after the final byte
  normsq = per-row sum of squares — DVE fused scalar_tensor_tensor
           (mul+accum in one pass) and ACT Square+accum, split 38/26 to
           finish under the ingest window; dumps use stride-0 out APs so
           the junk writes cost 1 byte/partition (less SBUF contention
           with the ingest DMAs, which are the kernel's bottleneck)

Host: packs fp8 layouts, then combines per-core partials in f64:
  numer_i = sum_c numer_c[i]; |x_i|^2 = sum_c normsq_c[i];
  |m|^2 = sum_c |m8_c|^2; cos_i = numer_i / (|x_i| |m|) — scale-invariant
  in m, so the 2^-6 scaling and the n0 division drop out.
"""

import contextlib

import numpy as np

B = 8192
D = 4096
P = 128
NCORES = 8
DC = D // NCORES        # 512 cols per core
T = B // P              # 64 row tiles
NK = DC // P            # 4 col chunks
HB = B // 2             # rows per half
MS = 2.0 ** -6          # m scale (keeps s0 in fp8 range)
EPS = 1e-8
NRC = 4                 # x_rm ingest chunks (1MB each)
RT_PER_CHUNK = T // NRC
V_TILES = 36            # normsq tiles on DVE; rest on ACT


def _build_program():
    import concourse.bass as bass
    import concourse.mybir as mybir

    f32 = mybir.dt.float32
    bf16 = mybir.dt.bfloat16
    fp8 = mybir.dt.float8e4
    AOP = mybir.AluOpType
    AF = mybir.ActivationFunctionType
    DR = mybir.MatmulPerfMode.DoubleRow

    nc = bass.Bass(trn_type="TRN2", num_devices=NCORES)

    xrm_d = nc.dram_tensor("xrm", [P, T * DC], fp8, kind="ExternalInput")
    xt_d = nc.dram_tensor("xt", [P, NK * B], fp8, kind="ExternalInput")
    m0_d = nc.dram_tensor("m0", [P, T], fp8, kind="ExternalInput")
    out_num = nc.dram_tensor("out_num", [1, B], f32, kind="ExternalOutput")
    out_nrm = nc.dram_tensor("out_nrm", [P, T], f32, kind="ExternalOutput")
    out_m8p = nc.dram_tensor("out_m8p", [P, 128], fp8, kind="ExternalOutput")

    ctx = contextlib.ExitStack()
    sb = lambda name, shape, dt: ctx.enter_context(nc.sbuf_tensor(name, shape, dt))

    x_rm = sb("x_rm", [P, T * DC], fp8)
    xts = sb("xts", [P, NK * B], fp8)
    m0s = sb("m0s", [P, T], fp8)         # [p, a*32+t2] = mask0(row (2*t2+a)*128+p)
    m8pad = sb("m8pad", [P, 128], fp8)   # stationary slots: col k*32 = m[k*128+p]
    dumpV = sb("dumpV", [P, 1], fp8)
    dumpA = sb("dumpA", [P, 1], fp8)
    normsq = sb("normsq", [P, T], f32)
    m16row = sb("m16row", [1, DC], bf16)
    one1 = sb("one1", [1, 1], bf16)
    nsp = sb("nsp", [1, B], f32)

    pt = ctx.enter_context(nc.psum_tensor("pt", [P, 4096]))

    sem = lambda name: ctx.enter_context(nc.semaphore(name))
    dxr = [sem(f"dxr{i}") for i in range(NRC)]
    dxt = [[[sem(f"dxt{h}_{g}_{jh}") for jh in range(2)] for g in range(2)] for h in range(2)]
    sm0 = sem("sm0")
    s_pe = sem("s_pe")
    s_cast = sem("s_cast")
    s_tr = sem("s_tr")
    s_m8 = sem("s_m8")
    s_nA0 = sem("s_nA0")
    s_nA1 = sem("s_nA1")
    s_nB0 = sem("s_nB0")
    s_nB1 = sem("s_nB1")
    s_spA = sem("s_spA")
    s_spB = sem("s_spB")
    s_spB1 = sem("s_spB1")
    s_nsV = sem("s_nsV")
    s_nsA = sem("s_nsA")
    s_out = sem("s_out")

    xrm3 = x_rm.rearrange("p (t c) -> p t c", c=DC)
    xt6 = xts.rearrange("p (g h j k c) -> p g h j k c", g=2, h=2, j=8, k=2)
    m0d = m0s.rearrange("p (a t) -> p a t", a=2)

    def rm_chunk(q):
        return slice(q * (T * DC // NRC), (q + 1) * (T * DC // NRC))

    def xt_sl(h, g, part=None):
        base = (g * 2 + h) * HB * 2
        if part is None:
            return slice(base, base + HB * 2)       # full 2MB block
        if part == 0:
            return slice(base, base + 7 * 1024)     # j-slots 0..6
        return slice(base + 7 * 1024, base + HB * 2)  # j-slot 7

    with nc.Block() as block:

        @block.sync
        def _(sync):
            # SP queue: even x_rm chunks, then xT (h, g=0)
            for q in (0, 2):
                sync.dma_start(x_rm[:, rm_chunk(q)], xrm_d[:, rm_chunk(q)]).then_inc(dxr[q], 16)
            sync.dma_start(xts[:, xt_sl(0, 0)],
                           xt_d[:, xt_sl(0, 0)]).then_inc(dxt[0][0][0], 16)
            for part in (0, 1):
                sync.dma_start(xts[:, xt_sl(1, 0, part)],
                               xt_d[:, xt_sl(1, 0, part)]).then_inc(dxt[1][0][part], 16)
            sync.wait_ge(s_spA, 2)
            sync.dma_start(out_num[:, 0:HB], nsp[:, 0:HB]).then_inc(s_out, 16)
            sync.wait_ge(s_spB, 2)
            sync.dma_start(out_num[:, HB : HB + 3584], nsp[:, HB : HB + 3584]).then_inc(s_out, 16)
            sync.wait_ge(s_spB1, 2)
            sync.dma_start(out_num[:, HB + 3584 : B], nsp[:, HB + 3584 : B]).then_inc(s_out, 16)
            sync.wait_ge(s_out, 80)

        @block.scalar
        def _(sc):
            # ACT queue: mask + odd x_rm chunks + xT (h, g=1)
            sc.dma_start(m0s[:, :], m0_d[:, :]).then_inc(sm0, 16)
            for q in (1, 3):
                sc.dma_start(x_rm[:, rm_chunk(q)], xrm_d[:, rm_chunk(q)]).then_inc(dxr[q], 16)
            sc.dma_start(xts[:, xt_sl(0, 1)],
                         xt_d[:, xt_sl(0, 1)]).then_inc(dxt[0][1][0], 16)
            for part in (0, 1):
                sc.dma_start(xts[:, xt_sl(1, 1, part)],
                             xt_d[:, xt_sl(1, 1, part)]).then_inc(dxt[1][1][part], 16)
            # normsq tiles V_TILES..V_TILES+13, then the cast, then the rest
            for t in range(V_TILES, V_TILES + 16):
                sc.wait_ge(dxr[t // RT_PER_CHUNK], 16)
                sc.activation(dumpA[:, 0:1].to_broadcast((P, DC)), xrm3[:, t, :],
                              AF.Square,
                              accum_out=normsq[:, t : t + 1]).then_inc(s_nsA, 1)
            sc.wait_ge(s_pe, 1)
            sc.activation(m16row[:, :], pt[0:1, 0:DC], AF.Copy, scale=MS).then_inc(s_cast, 1)
            for t in range(V_TILES + 16, T):
                sc.activation(dumpA[:, 0:1].to_broadcast((P, DC)), xrm3[:, t, :],
                              AF.Square,
                              accum_out=normsq[:, t : t + 1]).then_inc(s_nsA, 1)
            # normsq + m8pad outputs on the ACT queue (parallel with SP's out_num)
            sc.wait_ge(s_nsV, V_TILES)
            sc.dma_start(out_nrm[:, :], normsq[:, :]).then_inc(s_out, 16)
            sc.dma_start(out_m8p[:, :], m8pad[:, :]).then_inc(s_out, 16)
            # numer spills
            sc.wait_ge(s_nA0, 1)
            sc.copy(nsp[0:1, 2048:4096], pt[0:1, 2048:4096]).then_inc(s_spA, 1)
            sc.wait_ge(s_nB0, 1)
            sc.copy(nsp[0:1, HB + 1792 : HB + 3584], pt[0:1, 1792:3584]).then_inc(s_spB, 1)
            sc.wait_ge(s_nB1, 1)
            sc.copy(nsp[0:1, HB + 3840 : B], pt[0:1, 3840:4096]).then_inc(s_spB1, 1)

        @block.vector
        def _(ve):
            for t in range(V_TILES):
                ve.wait_ge(dxr[t // RT_PER_CHUNK], 16)
                nc.vector.scalar_tensor_tensor(
                    dumpV[:, 0:1].to_broadcast((P, DC)), xrm3[:, t, :], 1.0,
                    xrm3[:, t, :], AOP.mult, AOP.mult,
                    accum_out=normsq[:, t : t + 1],
                ).then_inc(s_nsV, 1)
                if t == 20:
                    # m spread: psum [128,4] -> fp8 stationary slots
                    ve.wait_ge(s_tr, 1)
                    nc.vector.tensor_copy(m8pad[:, 0:97:32], pt[:, 4092:4096]).then_inc(s_m8, 1)
            ve.wait_ge(s_nA0, 1)
            nc.vector.tensor_copy(nsp[0:1, 0:2048], pt[0:1, 0:2048]).then_inc(s_spA, 1)
            ve.wait_ge(s_nB0, 1)
            nc.vector.tensor_copy(nsp[0:1, HB : HB + 1792], pt[0:1, 0:1792]).then_inc(s_spB, 1)
            ve.wait_ge(s_nB1, 1)
            nc.vector.tensor_copy(nsp[0:1, HB + 3584 : HB + 3840], pt[0:1, 3584:3840]).then_inc(s_spB1, 1)

        @block.gpsimd
        def _(gp):
            gp.memset(one1[:, :], 1.0)

        @block.tensor
        def _(pe):
            import concourse.mybir as mybir
            pe.wait_ge(sm0, 16)
            # s0: DoubleRow over row-tile pairs -> psum[0:1, 0:512]
            for t2 in range(T // 2):
                pe.wait_ge(dxr[(2 * t2) // RT_PER_CHUNK], 16)
                mm = nc.tensor.matmul(
                    pt[0:1, 0:DC],
                    m0d[:, :, t2 : t2 + 1],
                    xrm3[:, 2 * t2 : 2 * t2 + 2, :],
                    start=(t2 == 0), stop=(t2 == T // 2 - 1),
                    perf_mode=mybir.MatmulPerfMode.DoubleRow,
                )
                if t2 == T // 2 - 1:
                    mm.then_inc(s_pe, 1)
            # spread m16row chunks onto partitions: K=1 matmuls vs ones
            pe.wait_ge(s_cast, 1)
            for k in range(NK):
                mm = nc.tensor.matmul(
                    pt[:, 4092 + k : 4093 + k],
                    m16row[0:1, k * P : (k + 1) * P],
                    one1[0:1, 0:1],
                    start=True, stop=True,
                )
                if k == NK - 1:
                    mm.then_inc(s_tr, 1)
            pe.wait_ge(s_m8, 1)
            # numer: DoubleRow, chasing ingest per (h, jh) quarter; psum quarter
            # is spilled before the same quarter of the next half reuses it
            pe.wait_ge(dxt[0][0][0], 16)
            pe.wait_ge(dxt[0][1][0], 16)
            for j in range(8):
                for g in range(2):
                    mm = nc.tensor.matmul(
                        pt[0:1, j * DC : (j + 1) * DC],
                        m8pad[:, g * 64 : g * 64 + 33 : 32],
                        xt6[:, g, 0, j, :, :],
                        start=(g == 0), stop=(g == 1),
                        perf_mode=mybir.MatmulPerfMode.DoubleRow,
                    )
                    if g == 1 and j == 7:
                        mm.then_inc(s_nA0, 1)
            # real streaming warm-keepers: plain-mode 512-col matmuls into the
            # partition-32 psum scratch keep the PE p-state up through the
            # data gap before half B (xT half-A data is resident)
            for i in range(20):
                nc.tensor.matmul(
                    pt[32:33, (i % 8) * DC : (i % 8 + 1) * DC],
                    m8pad[:, 0:1],
                    xt6[:, 0, 0, i % 8, 0, :],
                    start=True, stop=True,
                )
            pe.wait_ge(s_spA, 2)
            for part, jrange in ((0, range(7)), (1, range(7, 8))):
                pe.wait_ge(dxt[1][0][part], 16)
                pe.wait_ge(dxt[1][1][part], 16)
                for j in jrange:
                    for g in range(2):
                        mm = nc.tensor.matmul(
                            pt[0:1, j * DC : (j + 1) * DC],
                            m8pad[:, g * 64 : g * 64 + 33 : 32],
                            xt6[:, g, 1, j, :, :],
                            start=(g == 0), stop=(g == 1),
                            perf_mode=mybir.MatmulPerfMode.DoubleRow,
                        )
                        if g == 1 and j == jrange[-1]:
                            mm.then_inc(s_nB0 if part == 0 else s_nB1, 1)

    ctx.close()
    return nc


_PROGRAM = None
LAST_RESULT = None  # BassKernelResults of the most recent run (for profiling)


def _host_inputs(labels, datas):
    import ml_dtypes

    fp8 = ml_dtypes.float8_e4m3
    labels = np.asarray(labels, dtype=np.float32)
    datas = np.asarray(datas, dtype=np.float32)

    mask0 = (labels[:, 0] >= labels[:, 1]).astype(np.float32)  # argmax==0
    x8 = datas.astype(fp8)

    # m0 pair layout: [p, a*32+t2] = mask0[(2*t2+a)*128+p]
    mt = mask0.reshape(T, P)
    m0 = np.empty((P, T), dtype=np.float32)
    half = T // 2
    m0[:, 0:half] = mt[0::2].T
    m0[:, half:T] = mt[1::2].T
    m0 = np.ascontiguousarray(m0).astype(fp8)

    in_maps = []
    for c in range(NCORES):
        xc = x8[:, c * DC : (c + 1) * DC]                       # [8192, 512] fp8
        x_rm = np.ascontiguousarray(
            xc.reshape(T, P, DC).transpose(1, 0, 2)).reshape(P, T * DC)
        xt = np.ascontiguousarray(
            xc.T.reshape(2, 2, P, 2, 8, 512).transpose(2, 0, 3, 4, 1, 5)
        ).reshape(P, NK * B)
        in_maps.append({"xrm": x_rm, "xt": xt, "m0": m0})
    return in_maps, mask0


def _host_finish(results, mask0):
    mask0 = mask0.astype(np.float64)
    mask1 = 1.0 - mask0
    n0 = float(mask0.sum())
    n1 = float(mask1.sum())

    numer = np.zeros(B)
    normsq = np.zeros(B)
    msq = 0.0
    for c in range(NCORES):
        r = results[c]
        numer += np.asarray(r["out_num"], dtype=np.float64).reshape(-1)
        normsq += np.asarray(r["out_nrm"], dtype=np.float64).T.reshape(-1)
        m8p = np.asarray(r["out_m8p"]).astype(np.float64)
        for k in range(NK):
            msq += float((m8p[:, k * 32] ** 2).sum())

    if n0 > 0.0:
        xnorm = np.maximum(np.sqrt(normsq), EPS)
        mnorm = max(np.sqrt(msq), EPS * MS * max(n0, 1.0))
        q = np.abs(numer) / (xnorm * mnorm)
        sim = 1.0 - float((mask0 * q).sum()) / n0
        dif = float((mask1 * q).sum()) / n1 if n1 > 0.0 else 0.0
    else:
        sim = 0.0
        dif = 0.0

    sim = np.float32(sim)
    dif = np.float32(dif)
    return (np.float32(sim + dif), sim, dif)


def kernel(labels, datas):
    global _PROGRAM, LAST_RESULT
    from concourse.bass_utils import run_bass_kernel_spmd

    in_maps, mask0 = _host_inputs(labels, datas)
    if _PROGRAM is None:
        _PROGRAM = _build_program()
    res = run_bass_kernel_spmd(_PROGRAM, in_maps, list(range(NCORES)))
    LAST_RESULT = res
    return _host_finish(res.results, mask0)



# revision 30
# speedup vs baseline: 1.0949x; 1.0949x over previous
"""Trainium2 Bass kernel for nn_Diff_Label01_Loss (masked cosine-similarity loss).

Contract: kernel(labels, datas) takes FULL inputs (labels [8192,2] f32,
datas [8192,4096] f32), returns (total_loss, sim_loss, differ_loss).

Strategy — shard D (columns) across the 8 cores; NO collective:
  Core c owns cols [c*512, (c+1)*512) of datas, in TWO fp8 layouts
  (8.4MB/core total):
    x_rm [128, 64, 512]   row-tiles (partition p of tile t = row t*128+p)
    xT   [128, 2, 2, 8, 2, 512]  [p, g, h, j, kt, c] = x[h*4096+j*512+c,
                                  (2g+kt)*128+p] — pair-interleaved for
                                  DoubleRow fp8 matmuls
  s0_c   = masked column sum of the core's slice — PE DoubleRow matmuls,
           mask-pair stationary, accumulated in psum[0:1, 0:512]
  m8_c   = fp8(bf16(s0_c * 2^-6)) — ACT cast to bf16, four K=1 matmuls
           spread it onto partitions, ACT copies into pair-layout slots
  numer  = x_slice @ m8_c — PE DoubleRow into psum[0:1, 0:4096]; half B
           reuses the row after half A's spill (DR matmuls to psum
           partition 32/64 fail ISA checks); spills split DVE/ACT
  normsq = per-row sum of squares — split across DVE (scalar_tensor_tensor),
           ACT (Square activation) and GPSIMD (scalar_tensor_tensor),
           28/20/16 tiles, all chasing the x_rm ingest chunks

  The PE runs junk matmuls on garbage SBUF during the ~10us DMA-issue /
  preamble window so the HAM clock gate is already at full rate when the
  real DoubleRow stream starts.

Host: packs fp8 layouts, then combines per-core partials in f64:
  numer_i = sum_c numer_c[i]; |x_i|^2 = sum_c normsq_c[i];
  |m|^2 = sum_c |m8_c|^2; cos_i = numer_i / (|x_i| |m|) — scale-invariant
  in m, so the 2^-6 scaling and the n0 division drop out.
"""

import contextlib

import numpy as np

B = 8192
D = 4096
P = 128
NCORES = 8
DC = D // NCORES        # 512 cols per core
T = B // P              # 64 row tiles
NK = DC // P            # 4 col chunks
HB = B // 2             # rows per half
MS = 2.0 ** -6          # m scale (keeps s0 in fp8 range)
EPS = 1e-8
# x_rm ingest chunks: tile ranges [lo, hi) and (DVE, ACT) normsq split.
# Chunk 0 is split 4/12 tiles so the vector engines start ~2us earlier.
CHUNKS = [
    (0, 4, (3, 1)),      # 0a: 0.25MB, SP ring first
    (4, 16, (7, 5)),     # 0b: 0.75MB, SP
    (16, 32, (10, 6)),   # 1:  1MB, ACT ring
    (32, 48, (9, 7)),    # 2:  1MB, SP
    (48, 64, (9, 7)),    # 3:  1MB, ACT
]
NV_TOT = sum(c[2][0] for c in CHUNKS)
NA_TOT = sum(c[2][1] for c in CHUNKS)


def _build_program():
    import concourse.bass as bass
    import concourse.mybir as mybir

    f32 = mybir.dt.float32
    bf16 = mybir.dt.bfloat16
    fp8 = mybir.dt.float8e4
    AOP = mybir.AluOpType
    AF = mybir.ActivationFunctionType
    DR = mybir.MatmulPerfMode.DoubleRow

    nc = bass.Bass(trn_type="TRN2", num_devices=NCORES)

    xrm_d = nc.dram_tensor("xrm", [P, T * DC], fp8, kind="ExternalInput")
    xt_d = nc.dram_tensor("xt", [P, NK * B], fp8, kind="ExternalInput")
    m0_d = nc.dram_tensor("m0", [P, T], fp8, kind="ExternalInput")
    out_num = nc.dram_tensor("out_num", [1, B], f32, kind="ExternalOutput")
    out_nrm = nc.dram_tensor("out_nrm", [P, T], f32, kind="ExternalOutput")
    out_m8p = nc.dram_tensor("out_m8p", [P, 128], fp8, kind="ExternalOutput")

    ctx = contextlib.ExitStack()
    sb = lambda name, shape, dt: ctx.enter_context(nc.sbuf_tensor(name, shape, dt))

    x_rm = sb("x_rm", [P, T * DC], fp8)
    xts = sb("xts", [P, NK * B], fp8)
    m0s = sb("m0s", [P, T], fp8)         # [p, a*32+t2] = mask0(row (2*t2+a)*128+p)
    m8pad = sb("m8pad", [P, 128], fp8)   # stationary slots: col k*32 = m[k*128+p]
    dumpV = sb("dumpV", [P, 1], fp8)
    dumpA = sb("dumpA", [P, 1], fp8)
    dumpG = sb("dumpG", [P, 1], fp8)
    normsq = sb("normsq", [P, T], f32)
    m16row = sb("m16row", [1, DC], bf16)
    one1 = sb("one1", [1, 1], bf16)
    nsp = sb("nsp", [1, B], f32)         # numer row
    junkb = sb("junkb", [1, 1024], bf16)  # never written; junk warmup reads

    pt = ctx.enter_context(nc.psum_tensor("pt", [P, 4096]))

    sem = lambda name: ctx.enter_context(nc.semaphore(name))
    dxr = [sem(f"dxr{i}") for i in range(len(CHUNKS))]
    dxt = {(h, g): sem(f"dxt{h}{g}") for h in range(2) for g in range(2)}
    sm0 = sem("sm0")
    s_pe = sem("s_pe")
    s_cast = sem("s_cast")
    s_tr = sem("s_tr")
    s_m8 = sem("s_m8")
    s_hA = sem("s_hA")
    s_hB = sem("s_hB")
    s_spA = sem("s_spA")
    s_spB = sem("s_spB")
    s_nsV = sem("s_nsV")
    s_nsA = sem("s_nsA")
    s_nsG = sem("s_nsG")
    s_out = sem("s_out")

    xrm3 = x_rm.rearrange("p (t c) -> p t c", c=DC)
    xt6 = xts.rearrange("p (g h j k c) -> p g h j k c", g=2, h=2, j=8, k=2)
    m0d = m0s.rearrange("p (a t) -> p a t", a=2)

    def rm_chunk(q):
        lo, hi, _ = CHUNKS[q]
        return slice(lo * DC, hi * DC)

    def xt_sl(h, g):
        base = (g * 2 + h) * HB * 2
        return slice(base, base + HB * 2)   # full 1MB block

    # tile ownership within a chunk: DVE first, ACT rest
    def tiles_of(c, eng):
        lo, hi, (nv, na) = CHUNKS[c]
        if eng == "V":
            return range(lo, lo + nv)
        return range(lo + nv, hi)

    with nc.Block() as block:

        @block.sync
        def _(sync):
            sync.dma_start(x_rm[:, rm_chunk(0)], xrm_d[:, rm_chunk(0)]).then_inc(dxr[0], 16)
            sync.dma_start(x_rm[:, rm_chunk(1)], xrm_d[:, rm_chunk(1)]).then_inc(dxr[1], 16)
            sync.dma_start(x_rm[:, rm_chunk(3)], xrm_d[:, rm_chunk(3)]).then_inc(dxr[3], 16)
            sync.dma_start(xts[:, xt_sl(0, 0)], xt_d[:, xt_sl(0, 0)]).then_inc(dxt[(0, 0)], 16)
            sync.dma_start(xts[:, xt_sl(1, 0)], xt_d[:, xt_sl(1, 0)]).then_inc(dxt[(1, 0)], 16)
            # numer half A out as soon as both spill pieces land
            sync.wait_ge(s_spA, 2)
            sync.dma_start(out_num[:, 0:HB], nsp[:, 0:HB]).then_inc(s_out, 16)
            sync.wait_ge(s_spB, 2)
            sync.dma_start(out_num[:, HB:B], nsp[:, HB:B]).then_inc(s_out, 16)
            # normsq out when both engines are done
            sync.wait_ge(s_nsV, NV_TOT)
            sync.wait_ge(s_nsA, NA_TOT)
            sync.dma_start(out_nrm[:, :], normsq[:, :]).then_inc(s_out, 16)
            sync.wait_ge(s_out, 64)

        @block.scalar
        def _(sc):
            sc.dma_start(x_rm[:, rm_chunk(2)], xrm_d[:, rm_chunk(2)]).then_inc(dxr[2], 16)
            sc.dma_start(x_rm[:, rm_chunk(4)], xrm_d[:, rm_chunk(4)]).then_inc(dxr[4], 16)
            sc.dma_start(xts[:, xt_sl(0, 1)], xt_d[:, xt_sl(0, 1)]).then_inc(dxt[(0, 1)], 16)
            sc.dma_start(xts[:, xt_sl(1, 1)], xt_d[:, xt_sl(1, 1)]).then_inc(dxt[(1, 1)], 16)

            def act_tile(t):
                sc.activation(dumpA[:, 0:1].to_broadcast((P, DC)), xrm3[:, t, :],
                              AF.Square,
                              accum_out=normsq[:, t : t + 1]).then_inc(s_nsA, 1)

            # 11 tiles before the cast (lands ~when s0 completes)
            sc.wait_ge(dxr[0], 16)
            for t in tiles_of(0, "A"):
                act_tile(t)
            sc.wait_ge(dxr[1], 16)
            for t in tiles_of(1, "A"):
                act_tile(t)
            sc.wait_ge(dxr[2], 16)
            for t in list(tiles_of(2, "A"))[:5]:
                act_tile(t)
            # m dance: cast s0 -> bf16 row; after PE spreads it, pack fp8 slots
            sc.wait_ge(s_pe, 1)
            sc.activation(m16row[:, :], pt[0:1, 0:DC], AF.Copy, scale=MS).then_inc(s_cast, 1)
            sc.wait_ge(s_tr, 1)
            sc.copy(m8pad[:, 0:97:32], pt[:, 4092:4096]).then_inc(s_m8, 1)
            sc.dma_start(out_m8p[:, :], m8pad[:, :]).then_inc(s_out, 16)
            for t in list(tiles_of(2, "A"))[5:]:
                act_tile(t)
            sc.wait_ge(dxr[3], 16)
            for t in list(tiles_of(3, "A"))[:3]:
                act_tile(t)
            # numer half A spill piece (DVE takes the other half)
            sc.wait_ge(s_hA, 1)
            sc.copy(nsp[0:1, 2048:HB], pt[0:1, 2048:4096]).then_inc(s_spA, 1)
            for t in list(tiles_of(3, "A"))[3:]:
                act_tile(t)
            sc.wait_ge(dxr[4], 16)
            for t in list(tiles_of(4, "A"))[:4]:
                act_tile(t)
            sc.wait_ge(s_hB, 1)
            sc.copy(nsp[0:1, HB + 2048 : B], pt[0:1, 2048:4096]).then_inc(s_spB, 1)
            for t in list(tiles_of(4, "A"))[4:]:
                act_tile(t)

        @block.vector
        def _(ve):
            def dve_tile(t):
                nc.vector.scalar_tensor_tensor(
                    dumpV[:, 0:1].to_broadcast((P, DC)), xrm3[:, t, :], 1.0,
                    xrm3[:, t, :], AOP.mult, AOP.mult,
                    accum_out=normsq[:, t : t + 1],
                ).then_inc(s_nsV, 1)

            for c in range(3):
                ve.wait_ge(dxr[c], 16)
                for t in tiles_of(c, "V"):
                    dve_tile(t)
            ve.wait_ge(dxr[3], 16)
            tl = list(tiles_of(3, "V"))
            for t in tl[:5]:
                dve_tile(t)
            # numer half A spill piece; ACT takes the other half
            ve.wait_ge(s_hA, 1)
            nc.vector.tensor_copy(nsp[0:1, 0:2048], pt[0:1, 0:2048]).then_inc(s_spA, 1)
            for t in tl[5:]:
                dve_tile(t)
            ve.wait_ge(dxr[4], 16)
            tl4 = list(tiles_of(4, "V"))
            for t in tl4[:4]:
                dve_tile(t)
            # numer half B spill piece
            ve.wait_ge(s_hB, 1)
            nc.vector.tensor_copy(nsp[0:1, HB : HB + 2048], pt[0:1, 0:2048]).then_inc(s_spB, 1)
            for t in tl4[4:]:
                dve_tile(t)
        @block.gpsimd
        def _(gp):
            gp.memset(one1[:, :], 1.0)
            gp.dma_start(m0s[:, :], m0_d[:, :]).then_inc(sm0, 16)

        @block.tensor
        def _(pe):
            # HAM warmup: junk matmuls on garbage SBUF while DMA issues/preamble
            # run; keeps the PE clock gate at full rate for the real stream.
            for _ in range(16):
                nc.tensor.matmul(
                    pt[64:65, 0:256], junkb[0:1, 0:1], junkb[0:1, 0:256],
                    start=True, stop=True,
                )
            # s0: DoubleRow over row-tile pairs -> psum[0:1, 0:512]
            pe.wait_ge(sm0, 16)
            chunk_start = {lo // 2: i for i, (lo, hi, _) in enumerate(CHUNKS)}
            for t2 in range(T // 2):
                if t2 in chunk_start:
                    pe.wait_ge(dxr[chunk_start[t2]], 16)
                mm = nc.tensor.matmul(
                    pt[0:1, 0:DC],
                    m0d[:, :, t2 : t2 + 1],
                    xrm3[:, 2 * t2 : 2 * t2 + 2, :],
                    start=(t2 == 0), stop=(t2 == T // 2 - 1),
                    perf_mode=DR,
                )
            mm.then_inc(s_pe, 1)
            # spread m16row chunks onto partitions: K=1 matmuls vs ones
            pe.wait_ge(s_cast, 1)
            for k in range(NK):
                mm = nc.tensor.matmul(
                    pt[:, 4092 + k : 4093 + k],
                    m16row[0:1, k * P : (k + 1) * P],
                    one1[0:1, 0:1],
                    start=True, stop=True,
                )
            mm.then_inc(s_tr, 1)
            # keep the PE clock warm while waiting for m8pad + xt arrival
            for _ in range(4):
                nc.tensor.matmul(
                    pt[64:65, 0:256], junkb[0:1, 0:1], junkb[0:1, 0:256],
                    start=True, stop=True,
                )
            pe.wait_ge(s_m8, 1)
            # numer: DoubleRow into psum[0:1, :]; half B reuses the same psum
            # row, so it waits until both half A spill pieces are out.
            # g-outer order so the two contraction halves accumulate per column.
            for h in range(2):
                pe.wait_ge(dxt[(h, 0)], 16)
                pe.wait_ge(dxt[(h, 1)], 16)
                if h == 1:
                    pe.wait_ge(s_spA, 2)
                for g in range(2):
                    for j in range(8):
                        mm = nc.tensor.matmul(
                            pt[0:1, j * DC : (j + 1) * DC],
                            m8pad[:, g * 64 : g * 64 + 33 : 32],
                            xt6[:, g, h, j, :, :],
                            start=(g == 0), stop=(g == 1),
                            perf_mode=DR,
                        )
                mm.then_inc(s_hA if h == 0 else s_hB, 1)

    ctx.close()
    return nc


_PROGRAM = None
LAST_RESULT = None  # BassKernelResults of the most recent run (for profiling)
LAST_PROBE = None   # probe columns of out_nrm (core 0) from the last run


def _host_inputs(labels, datas):
    import ml_dtypes

    fp8 = ml_dtypes.float8_e4m3
    labels = np.asarray(labels, dtype=np.float32)
    datas = np.asarray(datas, dtype=np.float32)

    mask0 = (labels[:, 0] >= labels[:, 1]).astype(np.float32)  # argmax==0
    x8 = datas.astype(fp8)

    # m0 pair layout: [p, a*32+t2] = mask0[(2*t2+a)*128+p]
    mt = mask0.reshape(T, P)
    m0 = np.empty((P, T), dtype=np.float32)
    half = T // 2
    m0[:, 0:half] = mt[0::2].T
    m0[:, half:T] = mt[1::2].T
    m0 = np.ascontiguousarray(m0).astype(fp8)

    in_maps = []
    for c in range(NCORES):
        xc = x8[:, c * DC : (c + 1) * DC]                       # [8192, 512] fp8
        x_rm = np.ascontiguousarray(
            xc.reshape(T, P, DC).transpose(1, 0, 2)).reshape(P, T * DC)
        xt = np.ascontiguousarray(
            xc.T.reshape(2, 2, P, 2, 8, 512).transpose(2, 0, 3, 4, 1, 5)
        ).reshape(P, NK * B)
        in_maps.append({"xrm": x_rm, "xt": xt, "m0": m0})
    return in_maps, mask0


def _host_finish(results, mask0):
    global LAST_PROBE
    mask0 = mask0.astype(np.float64)
    mask1 = 1.0 - mask0
    n0 = float(mask0.sum())
    n1 = float(mask1.sum())

    numer = np.zeros(B)
    normsq = np.zeros(B)
    msq = 0.0
    for c in range(NCORES):
        r = results[c]
        numer += np.asarray(r["out_num"], dtype=np.float64).reshape(-1)
        normsq += np.asarray(r["out_nrm"], dtype=np.float64).T.reshape(-1)
        m8p = np.asarray(r["out_m8p"]).astype(np.float64)
        for k in range(NK):
            msq += float((m8p[:, k * 32] ** 2).sum())

    if n0 > 0.0:
        xnorm = np.maximum(np.sqrt(normsq), EPS)
        mnorm = max(np.sqrt(msq), EPS * MS * max(n0, 1.0))
        q = np.abs(numer) / (xnorm * mnorm)
        sim = 1.0 - float((mask0 * q).sum()) / n0
        dif = float((mask1 * q).sum()) / n1 if n1 > 0.0 else 0.0
    else:
        sim = 0.0
        dif = 0.0

    sim = np.float32(sim)
    dif = np.float32(dif)
    return (np.float32(sim + dif), sim, dif)


def kernel(labels, datas):
    global _PROGRAM, LAST_RESULT
    from concourse.bass_utils import run_bass_kernel_spmd

    in_maps, mask0 = _host_inputs(labels, datas)
    if _PROGRAM is None:
        _PROGRAM = _build_program()
    res = run_bass_kernel_spmd(_PROGRAM, in_maps, list(range(NCORES)))
    LAST_RESULT = res
    return _host_finish(res.results, mask0)
